# revision 28
# baseline (speedup 1.0000x reference)
"""TRN2 Bass kernel for nn_MultiHeadAttention_78056735637728.

8-way data parallel over batch (B=8, one batch element per NeuronCore).
Host side: the padding mask m is applied exactly by gathering the unmasked
kv positions; k/v are gathered and zero-padded to a multiple of 128 and a
per-position bias (-200 pre-softmax) kills the pad rows.

Device kernel (per core). PE cost on this target is free-columns only
(independent of K/M), so every matmul is shaped to put 128 useful rows on
the PE:
  - Q/K/V projections in bf16 (inputs and weights bf16, PSUM fp32 out).
  - logits.T (kv j on partitions, queries i free) via row-packed K=32
    f32r matmuls, two heads per [128,1024] PSUM tile.
  - P = exp(logits/sqrt(32) + kvb[j]) split across three engines:
    ScalarE exact exp (PSUM->bf16), DVE/GpSimd via the Schraudolph
    int16-bitcast-bf16 trick (~+-3.3% on individual weights, which the
    softmax ratio and averaging over ~1e3 kv positions wash out).
    GpSimd cannot read PSUM, so its tiles get a small HWDGE PSUM->SBUF
    copy first.
  - PV with P as lhsT: out[q=128, 33] per (head, j, subtile) where col 32
    is the ones-column denominator; bf16 rhs makes these 33-cycle matmuls.
    One accumulation bank per (chunk, head-pair); single start=True on the
    first write (PSUM zero-regions are 2KB).
  - normalize on DVE: strided reciprocal of the den columns + one
    broadcast scalar_tensor_tensor into natural-layout O (bf16).
  - PE transpose (bf16 identity) of O back to feature-major, ScalarE
    PSUM->SBUF copy, then a K=128 output projection in bf16.
"""
import math
import sys
from contextlib import ExitStack

import numpy as np

for _p in ("/opt/trn_rl_repo", "/root/.axon_site/_ro/trn_rl_repo"):
    import os as _os
    if _os.path.isdir(_p) and _p not in sys.path:
        sys.path.insert(0, _p)
        break

import ml_dtypes  # noqa: E402

import concourse.bass as bass  # noqa: E402
import concourse.tile as tile  # noqa: E402
from concourse import bacc, bass_utils, mybir  # noqa: E402
from concourse._compat import with_exitstack  # noqa: E402

F32 = mybir.dt.float32
F32R = mybir.dt.float32r
BF16 = mybir.dt.bfloat16
I16 = mybir.dt.int16
B = 8
S = 2048
D = 256
H = 8
DH = 32
PDIM = 128
N_CORES = 8

SCALE = 1.0 / math.sqrt(DH)
LOG2E = 1.4426950408889634
SCHRA_A = 128.0 * LOG2E           # int16 bf16-bitcast exp: bits = A*x + B
SCHRA_B = 16256.0 - 5.51          # 127<<7, centered for minimax rel err
PAD_BIAS = -200.0                 # pad kv bias: exp underflow / tiny negative

# exp-engine split weights (ScalarE exact, DVE schraudolph, Pool schraudolph
# behind a PSUM->SBUF copy)
W_SCALAR, W_DVE, W_POOL = 0.52, 0.48, 0.0

IN_NAMES = ["qt", "kt", "vt", "wb", "fb", "bvrow"]
_BF16_INPUTS = {"qt", "kt", "vt", "wb", "bvrow"}


def _exp_engines(n):
    """Weighted deficit round-robin over (S, D, P)."""
    w = {k: v for k, v in
         (("S", W_SCALAR), ("D", W_DVE), ("P", W_POOL)) if v > 0}
    defi = {k: 0.0 for k in w}
    out = []
    for _ in range(n):
        for k in w:
            defi[k] += w[k]
        pick = max(defi, key=lambda k: defi[k])
        defi[pick] -= 1.0
        out.append(pick)
    return out


@with_exitstack
def _mha_kernel(ctx: ExitStack, tc: tile.TileContext, outs, ins, SKV, S=S):
    nc = tc.nc
    (i_qt, i_kt, i_vt, i_wb, i_fb, i_bvrow) = ins
    o_ot = outs[0]

    NJ = SKV // PDIM
    NQC = S // 512
    assert S % 512 == 0 and SKV % PDIM == 0

    def chunks(total, step=512):
        out = []
        c = 0
        while c < total:
            w = min(step, total - c)
            out.append((c, w))
            c += w
        return out

    consts = ctx.enter_context(tc.tile_pool(name="consts", bufs=1))

    def load2(ap_dram, cols, dt, eng, step=512):
        ts = []
        for b in range(2):
            t = consts.tile([PDIM, cols], dt, name=f"{ap_dram.name}_sb{b}")
            for c0, w in chunks(cols, step):
                eng.dma_start(t[:, c0:c0 + w],
                              ap_dram[b * PDIM:(b + 1) * PDIM, c0:c0 + w])
            ts.append(t)
        return ts

    def load_bias(ap_dram, name, eng):
        t = consts.tile([PDIM, 2], F32, name=name)
        eng.dma_start(t[:], ap_dram.rearrange("(b p) -> p b", p=PDIM))
        return t

    # --- prologue loads, ordered by first use.  SP: K/V path.
    # Activation queue: biases + Q path.  Weights arrive as one [128, 512]
    # block-concat DMA each; qt/vt split so chunk-0 columns land first. ---
    wbund = consts.tile([PDIM, 4 * 512 + PDIM], BF16, name="wbund")
    wq_sb = [wbund[:, 0 * 512 + ib * D:0 * 512 + (ib + 1) * D]
             for ib in range(2)]
    wk_sb = [wbund[:, 1 * 512 + ib * D:1 * 512 + (ib + 1) * D]
             for ib in range(2)]
    wv_sb = [wbund[:, 2 * 512 + ib * D:2 * 512 + (ib + 1) * D]
             for ib in range(2)]
    wo_sb = [wbund[:, 3 * 512 + ib * D:3 * 512 + (ib + 1) * D]
             for ib in range(2)]
    ident = wbund[:, 2048:2048 + PDIM]
    fb = consts.tile([PDIM, 6 + 2 * NJ], F32, name="fb")
    nc.scalar.dma_start(fb[:], i_fb[:])
    bq_sb = fb[:, 0:2]
    bk_sb = fb[:, 2:4]
    bo_sb = fb[:, 4:6]
    kvba_sb = fb[:, 6:6 + NJ]
    kvbd_sb = fb[:, 6 + NJ:6 + 2 * NJ]

    acts = ctx.enter_context(tc.tile_pool(name="acts", bufs=1))
    kt_sb, vt_sb, qt_sb = [], [], []
    for b in range(2):
        kt_sb.append(acts.tile([PDIM, SKV], BF16, name=f"kt_sb{b}"))
        vt_sb.append(acts.tile([PDIM, SKV], BF16, name=f"vt_sb{b}"))
        qt_sb.append(acts.tile([PDIM, S], BF16, name=f"qt_sb{b}"))

    SK0 = min(512, SKV)
    nc.sync.dma_start(wbund[:, 512:1024], i_wb[:, 512:1024])      # wk
    for b in range(2):
        nc.sync.dma_start(kt_sb[b][:, 0:SK0],
                          i_kt[b * PDIM:(b + 1) * PDIM, 0:SK0])
    nc.sync.dma_start(wbund[:, 0:512], i_wb[:, 0:512])            # wq
    for b in range(2):
        nc.sync.dma_start(qt_sb[b][:, 0:512],
                          i_qt[b * PDIM:(b + 1) * PDIM, 0:512])
    nc.sync.dma_start(wbund[:, 1024:1536], i_wb[:, 1024:1536])    # wv
    for b in range(2):
        nc.sync.dma_start(vt_sb[b][:, 0:SK0],
                          i_vt[b * PDIM:(b + 1) * PDIM, 0:SK0])
    bvrow_sb = consts.tile([1, D], BF16, name="bvrow_sb")
    nc.sync.dma_start(bvrow_sb[:], i_bvrow[:])
    if SKV > SK0:
        for b in range(2):
            nc.sync.dma_start(kt_sb[b][:, SK0:SKV],
                              i_kt[b * PDIM:(b + 1) * PDIM, SK0:SKV])
    if SKV > SK0:
        for b in range(2):
            nc.sync.dma_start(vt_sb[b][:, SK0:SKV],
                              i_vt[b * PDIM:(b + 1) * PDIM, SK0:SKV])
    for b in range(2):
        nc.sync.dma_start(qt_sb[b][:, 512:S],
                          i_qt[b * PDIM:(b + 1) * PDIM, 512:S])
    nc.sync.dma_start(wbund[:, 1536:2048], i_wb[:, 1536:2048])    # wo
    nc.sync.dma_start(wbund[:, 2048:2048 + PDIM],
                      i_wb[:, 2048:2048 + PDIM])                  # ident
    onecol = consts.tile([1, PDIM], BF16, name="onecol")
    nc.gpsimd.memset(onecol[:], 1.0)
    # exp table load at t~0, off the critical path of the first real exp
    warm = consts.tile([PDIM, 1], F32, name="warm")
    nc.scalar.activation(warm[:], fb[:, 6:7],
                         mybir.ActivationFunctionType.Exp)

    QT = [acts.tile([PDIM, S], F32R, name=f"QT{b}") for b in range(2)]
    KT = [acts.tile([PDIM, SKV], F32R, name=f"KT{b}") for b in range(2)]
    # V natural layout per kv tile: [128 kv, 8 heads x (32 V | 1 ones)] bf16
    VNE = [acts.tile([PDIM, H * (DH + 1)], BF16, name=f"VNE{j}")
           for j in range(NJ)]
    for j in range(NJ):
        v3 = VNE[j][:].rearrange("p (h c) -> p h c", c=DH + 1)
        nc.gpsimd.memset(v3[:, :, DH:DH + 1], 1.0)

    # --- pools ---
    lt_pool = ctx.enter_context(
        tc.tile_pool(name="lt", bufs=3, space="PSUM"))      # 6 banks
    bank_a = ctx.enter_context(
        tc.tile_pool(name="bnka", bufs=1, space="PSUM"))    # 1 bank
    bank_b = ctx.enter_context(
        tc.tile_pool(name="bnkb", bufs=1, space="PSUM"))    # 1 bank
    p_pool = ctx.enter_context(tc.tile_pool(name="psb", bufs=8))
    onat_pool = ctx.enter_context(tc.tile_pool(name="onat", bufs=3))
    otsb_pool = ctx.enter_context(tc.tile_pool(name="otsb", bufs=2))
    rec_pool = ctx.enter_context(tc.tile_pool(name="recp", bufs=4))
    ft_pool = ctx.enter_context(tc.tile_pool(name="ftp", bufs=4))

    def lt_borrow(cols):
        t = lt_pool.tile([PDIM, 1024], F32, tag="lt", name="ltb")
        return t[:, 0:cols]

    def proj_qk_one(dst, w_sb, b_sb, x_sb, c0, w, ob, ps, eng="D"):
        for ib in range(2):
            nc.tensor.matmul(
                ps[:, 0:w],
                lhsT=w_sb[ib][:, ob * PDIM:(ob + 1) * PDIM],
                rhs=x_sb[ib][:, c0:c0 + w],
                start=(ib == 0), stop=(ib == 1),
            )
        if eng == "D":
            nc.vector.tensor_scalar_add(
                dst[ob][:, c0:c0 + w], ps[:, 0:w], b_sb[:, ob:ob + 1])
        else:
            nc.scalar.activation(
                dst[ob][:, c0:c0 + w], ps[:, 0:w],
                mybir.ActivationFunctionType.Identity,
                bias=b_sb[:, ob:ob + 1])

    def proj_v(j):
        ps = bank_b.tile([PDIM, 512], F32, tag="b", name="bbank")
        for ib in range(2):
            nc.tensor.matmul(
                ps[:, 0:D],
                lhsT=vt_sb[ib][:, j * PDIM:(j + 1) * PDIM],
                rhs=wv_sb[ib][:],
                start=(ib == 0), stop=False,
            )
        nc.tensor.matmul(ps[:, 0:D], lhsT=onecol[:], rhs=bvrow_sb[:],
                         start=False, stop=True)
        v3 = VNE[j][:].rearrange("p (h c) -> p h c", c=DH + 1)
        nc.scalar.activation(
            v3[:, :, 0:DH], ps[:, 0:D].rearrange("p (h c) -> p h c", c=DH),
            mybir.ActivationFunctionType.Copy)

    # K projection (bf16 inputs -> f32r KT); chunk 0 split across the two
    # single-bank pools so logits can start after ~one matmul pass; all other
    # projection chunks borrow lt-ring slots
    kc = chunks(SKV)
    c0, w = kc[0]
    proj_qk_one(KT, wk_sb, bk_sb, kt_sb, c0, w, 0,
                bank_a.tile([PDIM, 512], F32, tag="a", name="abank"))
    proj_qk_one(KT, wk_sb, bk_sb, kt_sb, c0, w, 1,
                bank_b.tile([PDIM, 512], F32, tag="b", name="bbank"), eng="S")
    proj_qk_one(QT, wq_sb, bq_sb, qt_sb, 0, 512, 0, lt_borrow(512))
    proj_qk_one(QT, wq_sb, bq_sb, qt_sb, 0, 512, 1, lt_borrow(512), eng="S")
    # remaining K chunks interleave into the first head-pair group
    deferred = []
    for c0, w in kc[1:]:
        for ob in range(2):
            deferred.append((c0, w, ob))

    exp_engines = _exp_engines(NQC * 4 * NJ)

    def tail_stage0(c, last=False):
        """normalized O_nat[c] -> transpose -> OT sbuf"""
        onat_lo, onat_hi = onat_tiles[c]
        otp = lt_pool.tile([PDIM, 1024], F32, tag="lt")
        otv = otp[:].bitcast(BF16)[:, 0:1024]
        for k in range(8):
            s, fh = k // 2, k % 2
            src_t = onat_lo if fh == 0 else onat_hi
            nc.tensor.matmul(
                otv[:, fh * 512 + s * PDIM:fh * 512 + (s + 1) * PDIM],
                lhsT=src_t[:, s * PDIM:(s + 1) * PDIM],
                rhs=ident, is_transpose=True,
                start=(k == 0), stop=(k == 7), skip_group_check=True)
        ot_sb = otsb_pool.tile([PDIM, 1024], BF16, tag="otsb")
        if last:
            nc.scalar.activation(ot_sb[:, 0:512], otv[:, 0:512],
                                 mybir.ActivationFunctionType.Copy)
            nc.vector.tensor_scalar_add(ot_sb[:, 512:1024],
                                        otv[:, 512:1024], 0.0)
        else:
            nc.scalar.activation(ot_sb[:], otv,
                                 mybir.ActivationFunctionType.Copy)
        return ot_sb

    def tail_proj(c, ot_sb, ob, last=False):
        ps = (bank_a.tile([PDIM, 512], F32, tag="a", name="abank") if ob == 0
              else lt_borrow(512))
        for fb in range(2):
            nc.tensor.matmul(
                ps[:],
                lhsT=wo_sb[fb][:, ob * PDIM:(ob + 1) * PDIM],
                rhs=ot_sb[:, fb * 512:(fb + 1) * 512],
                start=(fb == 0), stop=(fb == 1))
        ft = ft_pool.tile([PDIM, 512], F32, tag="ft")
        if last and ob == 1:
            nc.vector.tensor_scalar_add(ft[:], ps[:], bo_sb[:, ob:ob + 1])
        else:
            nc.scalar.activation(
                ft[:], ps[:], mybir.ActivationFunctionType.Identity,
                bias=bo_sb[:, ob:ob + 1])
        nc.sync.dma_start(
            o_ot[ob * PDIM:(ob + 1) * PDIM, c * 512:(c + 1) * 512], ft[:])

    onat_tiles = {}
    tail_ot = [None]
    eidx = 0
    for ic in range(NQC):
        onat_lo = onat_pool.tile([PDIM, 512], BF16, tag="onlo")
        onat_hi = onat_pool.tile([PDIM, 512], BF16, tag="onhi")
        onat_tiles[ic] = (onat_lo, onat_hi)
        i0 = ic * 512
        for hp in range(4):
            h0 = 2 * hp
            opv = (bank_a.tile([PDIM, 512], F32, tag="a", name="abank") if hp % 2 == 0
                   else bank_b.tile([PDIM, 512], F32, tag="b", name="bbank"))
            first_pv = [True]
            pvq = []

            def emit_pv(j, pt, last):
                for s in range(4):
                    for hh in range(2):
                        g = 2 * s + hh
                        nc.tensor.matmul(
                            opv[:, 33 * g:33 * g + 33],
                            lhsT=pt[:, hh * 512 + s * PDIM:
                                    hh * 512 + (s + 1) * PDIM],
                            rhs=VNE[j][:, 33 * (h0 + hh):33 * (h0 + hh) + 33],
                            start=first_pv[0], stop=last,
                            skip_group_check=True)
                        first_pv[0] = False

            for j in range(NJ):
                if hp == 1 and ic > 0:
                    if j == 0:
                        tail_ot[0] = tail_stage0(ic - 1)
                    elif j == min(2, NJ - 1):
                        tail_proj(ic - 1, tail_ot[0], 0)
                    elif j == min(4, NJ - 1):
                        tail_proj(ic - 1, tail_ot[0], 1)
                if hp == 3 and ic + 1 < NQC and j in (0, min(2, NJ - 1)):
                    ob = 0 if j == 0 else 1
                    proj_qk_one(
                        QT, wq_sb, bq_sb, qt_sb, (ic + 1) * 512, 512, ob,
                        bank_a.tile([PDIM, 512], F32, tag="a",
                                    name="abank") if ob == 0
                        else lt_borrow(512))
                if hp == 3 and ic == NQC - 1 and j == 0:
                    # last chunk: transpose+copy the first feature half
                    # (heads 0-3, normalized after hp1) on bank A now
                    otp_l = bank_a.tile([PDIM, 512], F32, tag="a",
                                        name="abank")
                    otv_l = otp_l[:].bitcast(BF16)
                    onat3lo, _ = onat_tiles[ic]
                    for s4 in range(4):
                        nc.tensor.matmul(
                            otv_l[:, s4 * PDIM:(s4 + 1) * PDIM],
                            lhsT=onat3lo[:, s4 * PDIM:(s4 + 1) * PDIM],
                            rhs=ident, is_transpose=True,
                            start=(s4 == 0), stop=False,
                            skip_group_check=True)
                    tail_ot[0] = otsb_pool.tile([PDIM, 1024], BF16,
                                                tag="otsb", name="otsb_t")
                if hp == 3 and ic == NQC - 1 and j == 1:
                    nc.scalar.activation(tail_ot[0][:, 0:512],
                                         otv_l[:, 0:512],
                                         mybir.ActivationFunctionType.Copy)
                # logits for the head pair into one [128, 1024] PSUM tile
                lt = lt_pool.tile([PDIM, 1024], F32, tag="lt", name="ltb")
                for hh, h in enumerate((h0, h0 + 1)):
                    t, bp = h // 4, DH * (h % 4)
                    nc.tensor.matmul(
                        lt[:, hh * 512:(hh + 1) * 512],
                        lhsT=KT[t][bp:bp + DH, j * PDIM:(j + 1) * PDIM],
                        rhs=QT[t][bp:bp + DH, i0:i0 + 512],
                        start=True, stop=True,
                        tile_position=(bp, 0),
                    )
                # exp on the assigned engine
                pt = p_pool.tile([PDIM, 1024], BF16, tag="pt")
                eng = exp_engines[eidx]
                eidx += 1
                if eng == "S":
                    nc.scalar.activation(
                        pt[:], lt[:], mybir.ActivationFunctionType.Exp,
                        bias=kvba_sb[:, j:j + 1], scale=SCALE)
                else:
                    nc.vector.tensor_scalar(
                        pt[:].bitcast(I16), lt[:], SCHRA_A * SCALE,
                        kvbd_sb[:, j:j + 1],
                        op0=mybir.AluOpType.mult, op1=mybir.AluOpType.add)
                if ic == 0 and hp == 0:
                    if deferred and j >= 2:
                        kcc0, kcw, kcob = deferred.pop(0)
                        proj_qk_one(KT, wk_sb, bk_sb, kt_sb, kcc0, kcw, kcob,
                                    lt_borrow(kcw))
                    proj_v(j)
                pvq.append((j, pt))
                if j >= 2:
                    jq, ptq = pvq.pop(0)
                    emit_pv(jq, ptq, last=False)
            while pvq:
                jq, ptq = pvq.pop(0)
                emit_pv(jq, ptq, last=(not pvq))
            # normalize: rec = 1/den (den at col 32 of each 33-group)
            og = opv[:, 0:264].rearrange("p (s h c) -> p s h c", h=2, c=33)
            rec = rec_pool.tile([PDIM, 8], F32, tag="rec")
            r3 = rec[:].rearrange("p (s h) -> p s h", h=2)
            nc.vector.reciprocal(r3.unsqueeze(-1), og[:, :, :, 32:33])
            on_t = onat_lo if hp < 2 else onat_hi
            on = on_t[:].rearrange("p (s f) -> p s f", f=PDIM)
            on = on[:, :, 64 * (hp % 2):64 * (hp % 2 + 1)]
            on = on.rearrange("p s (h c) -> p s h c", c=DH)
            nc.vector.scalar_tensor_tensor(
                on, og[:, :, :, 0:DH], 1.0,
                r3.unsqueeze(-1).broadcast_to((PDIM, 4, 2, DH)),
                op0=mybir.AluOpType.mult, op1=mybir.AluOpType.mult)
    _, onat3hi = onat_tiles[NQC - 1]
    for s4 in range(4):
        nc.tensor.matmul(
            otv_l[:, 512 + s4 * PDIM:512 + (s4 + 1) * PDIM],
            lhsT=onat3hi[:, s4 * PDIM:(s4 + 1) * PDIM],
            rhs=ident, is_transpose=True,
            start=False, stop=(s4 == 3), skip_group_check=True)
    nc.vector.tensor_scalar_add(tail_ot[0][:, 512:1024],
                                otv_l[:, 512:1024], 0.0)
    tail_proj(NQC - 1, tail_ot[0], 0, last=True)
    tail_proj(NQC - 1, tail_ot[0], 1, last=True)


_PROGRAM_CACHE = {}


def _make_program(SKV, S=S):
    nc = bacc.Bacc("TRN2", target_bir_lowering=False, debug=False,
                   enable_asserts=False, num_devices=1)
    NJ = SKV // PDIM
    shapes = dict(qt=(D, S), kt=(D, SKV), vt=(D, SKV),
                  wb=(PDIM, 2176), fb=(PDIM, 6 + 2 * NJ), bvrow=(1, D))
    in_aps = [nc.dram_tensor(k, shapes[k],
                             BF16 if k in _BF16_INPUTS else F32,
                             kind="ExternalInput").ap()
              for k in IN_NAMES]
    out_ap = nc.dram_tensor("ot", (D, S), F32, kind="ExternalOutput").ap()
    with tile.TileContext(nc) as tc:
        _mha_kernel(tc, [out_ap], in_aps, SKV=SKV, S=S)
    nc.compile()
    return nc


def _get_program(SKV):
    if SKV not in _PROGRAM_CACHE:
        _PROGRAM_CACHE[SKV] = _make_program(SKV)
    return _PROGRAM_CACHE[SKV]


def _prepare_in_maps(q, k, v, m, wq, bq, wk, bk, wv, bv, wo, bo):
    mask = np.asarray(m, np.float32).reshape(-1)
    keep = np.flatnonzero(mask == 0.0)
    skv = len(keep)
    assert skv > 0, "all kv positions masked"
    SKV = max(PDIM, ((skv + PDIM - 1) // PDIM) * PDIM)

    NJ = SKV // PDIM
    kvba = np.zeros(SKV, np.float32)
    kvba[skv:] = PAD_BIAS
    kvbd = (SCHRA_A * kvba + SCHRA_B).astype(np.float32)
    bf = ml_dtypes.bfloat16

    def blocks(wm):  # [256, 256] -> [128, 512] (two 128-row blocks side by side)
        a = np.asarray(wm, np.float32)
        return a.reshape(2, PDIM, D).transpose(1, 0, 2).reshape(PDIM, 2 * D)

    wbund = np.concatenate(
        [blocks(wq), blocks(wk), blocks(wv), blocks(wo),
         np.eye(PDIM, dtype=np.float32)], axis=1).astype(bf)
    fbund = np.concatenate(
        [np.asarray(bq, np.float32).reshape(PDIM, 2, order="F"),
         np.asarray(bk, np.float32).reshape(PDIM, 2, order="F"),
         np.asarray(bo, np.float32).reshape(PDIM, 2, order="F"),
         kvba.reshape(NJ, PDIM).T, kvbd.reshape(NJ, PDIM).T], axis=1)
    common = dict(
        wb=np.ascontiguousarray(wbund),
        fb=np.ascontiguousarray(fbund.astype(np.float32)),
        bvrow=np.asarray(bv, np.float32).reshape(1, D).astype(bf),
    )
    in_maps = []
    for b in range(B):
        kg = np.zeros((D, SKV), bf)
        vg = np.zeros((D, SKV), bf)
        kg[:, :skv] = np.asarray(k[b], np.float32).T[:, keep].astype(bf)
        vg[:, :skv] = np.asarray(v[b], np.float32).T[:, keep].astype(bf)
        in_maps.append(dict(
            qt=np.ascontiguousarray(np.asarray(q[b], np.float32).T.astype(bf)),
            kt=kg, vt=vg, **common))
    return in_maps, SKV


def _run(q, k, v, m, wq, bq, wk, bk, wv, bv, wo, bo, trace=False):
    in_maps, SKV = _prepare_in_maps(q, k, v, m, wq, bq, wk, bk, wv, bv, wo, bo)
    nc = _get_program(SKV)
    last_err = None
    for attempt in range(3):
        try:
            res = bass_utils.run_bass_kernel_spmd(
                nc, in_maps, core_ids=list(range(N_CORES)), trace=trace)
            break
        except Exception as e:  # transient device-unrecoverable states heal
            last_err = e        # on the next NEFF load; retry
    else:
        raise last_err
    out = np.stack([res.results[b]["ot"].T for b in range(B)], axis=0)
    return np.ascontiguousarray(out, np.float32), res


def kernel(q, k, v, m, wq, bq, wk, bk, wv, bv, wo, bo):
    out, _ = _run(q, k, v, m, wq, bq, wk, bk, wv, bv, wo, bo, trace=False)
    return out


# revision 30
# speedup vs baseline: 1.0241x; 1.0241x over previous
"""TRN2 Bass kernel for nn_MultiHeadAttention_78056735637728.

8-way data parallel over batch (B=8, one batch element per NeuronCore).
Host side: the padding mask m is applied exactly by gathering the unmasked
kv positions; k/v are gathered and zero-padded to a multiple of 128 and a
per-position bias (-200 pre-softmax) kills the pad rows.

Device kernel (per core). PE cost on this target is free-columns only
(independent of K/M), so every matmul is shaped to put 128 useful rows on
the PE:
  - Q/K/V projections in bf16 (inputs and weights bf16, PSUM fp32 out).
  - logits.T (kv j on partitions, queries i free) via row-packed K=32
    f32r matmuls, two heads per [128,1024] PSUM tile.
  - P = exp(logits/sqrt(32) + kvb[j]) split across three engines:
    ScalarE exact exp (PSUM->bf16), DVE/GpSimd via the Schraudolph
    int16-bitcast-bf16 trick (~+-3.3% on individual weights, which the
    softmax ratio and averaging over ~1e3 kv positions wash out).
    GpSimd cannot read PSUM, so its tiles get a small HWDGE PSUM->SBUF
    copy first.
  - PV with P as lhsT: out[q=128, 33] per (head, j, subtile) where col 32
    is the ones-column denominator; bf16 rhs makes these 33-cycle matmuls.
    One accumulation bank per (chunk, head-pair); single start=True on the
    first write (PSUM zero-regions are 2KB).
  - normalize on DVE: strided reciprocal of the den columns + one
    broadcast scalar_tensor_tensor into natural-layout O (bf16).
  - PE transpose (bf16 identity) of O back to feature-major, ScalarE
    PSUM->SBUF copy, then a K=128 output projection in bf16.
"""
import math
import sys
from contextlib import ExitStack

import numpy as np

for _p in ("/opt/trn_rl_repo", "/root/.axon_site/_ro/trn_rl_repo"):
    import os as _os
    if _os.path.isdir(_p) and _p not in sys.path:
        sys.path.insert(0, _p)
        break

import ml_dtypes  # noqa: E402

import concourse.bass as bass  # noqa: E402
import concourse.tile as tile  # noqa: E402
from concourse import bacc, bass_utils, mybir  # noqa: E402
from concourse._compat import with_exitstack  # noqa: E402

F32 = mybir.dt.float32
F32R = mybir.dt.float32r
BF16 = mybir.dt.bfloat16
I16 = mybir.dt.int16
B = 8
S = 2048
D = 256
H = 8
DH = 32
PDIM = 128
N_CORES = 8

SCALE = 1.0 / math.sqrt(DH)
LOG2E = 1.4426950408889634
SCHRA_A = 128.0 * LOG2E           # int16 bf16-bitcast exp: bits = A*x + B
SCHRA_B = 16256.0 - 5.51          # 127<<7, centered for minimax rel err
PAD_BIAS = -200.0                 # pad kv bias: exp underflow / tiny negative

# exp-engine split weights (ScalarE exact, DVE schraudolph, Pool schraudolph
# behind a PSUM->SBUF copy)
W_SCALAR, W_DVE, W_POOL = 0.5, 0.5, 0.0

IN_NAMES = ["qt", "kt", "vt", "wb", "fb", "bvrow"]
_BF16_INPUTS = {"qt", "kt", "vt", "wb", "bvrow"}


def _exp_engines(n):
    """Weighted deficit round-robin over (S, D, P)."""
    w = {k: v for k, v in
         (("S", W_SCALAR), ("D", W_DVE), ("P", W_POOL)) if v > 0}
    defi = {k: 0.0 for k in w}
    out = []
    for _ in range(n):
        for k in w:
            defi[k] += w[k]
        pick = max(defi, key=lambda k: defi[k])
        defi[pick] -= 1.0
        out.append(pick)
    return out


@with_exitstack
def _mha_kernel(ctx: ExitStack, tc: tile.TileContext, outs, ins, SKV, S=S):
    nc = tc.nc
    (i_qt, i_kt, i_vt, i_wb, i_fb, i_bvrow) = ins
    o_ot = outs[0]

    NJ = SKV // PDIM
    NQC = S // 512
    assert S % 512 == 0 and SKV % PDIM == 0

    def chunks(total, step=512):
        out = []
        c = 0
        while c < total:
            w = min(step, total - c)
            out.append((c, w))
            c += w
        return out

    consts = ctx.enter_context(tc.tile_pool(name="consts", bufs=1))

    def load2(ap_dram, cols, dt, eng, step=512):
        ts = []
        for b in range(2):
            t = consts.tile([PDIM, cols], dt, name=f"{ap_dram.name}_sb{b}")
            for c0, w in chunks(cols, step):
                eng.dma_start(t[:, c0:c0 + w],
                              ap_dram[b * PDIM:(b + 1) * PDIM, c0:c0 + w])
            ts.append(t)
        return ts

    def load_bias(ap_dram, name, eng):
        t = consts.tile([PDIM, 2], F32, name=name)
        eng.dma_start(t[:], ap_dram.rearrange("(b p) -> p b", p=PDIM))
        return t

    # --- prologue loads, ordered by first use.  SP: K/V path.
    # Activation queue: biases + Q path.  Weights arrive as one [128, 512]
    # block-concat DMA each; qt/vt split so chunk-0 columns land first. ---
    wbund = consts.tile([PDIM, 4 * 512 + PDIM], BF16, name="wbund")
    wq_sb = [wbund[:, 0 * 512 + ib * D:0 * 512 + (ib + 1) * D]
             for ib in range(2)]
    wk_sb = [wbund[:, 1 * 512 + ib * D:1 * 512 + (ib + 1) * D]
             for ib in range(2)]
    wv_sb = [wbund[:, 2 * 512 + ib * D:2 * 512 + (ib + 1) * D]
             for ib in range(2)]
    wo_sb = [wbund[:, 3 * 512 + ib * D:3 * 512 + (ib + 1) * D]
             for ib in range(2)]
    ident = wbund[:, 2048:2048 + PDIM]
    fb = consts.tile([PDIM, 6 + 2 * NJ], F32, name="fb")
    nc.scalar.dma_start(fb[:], i_fb[:])
    bq_sb = fb[:, 0:2]
    bk_sb = fb[:, 2:4]
    bo_sb = fb[:, 4:6]
    kvba_sb = fb[:, 6:6 + NJ]
    kvbd_sb = fb[:, 6 + NJ:6 + 2 * NJ]

    acts = ctx.enter_context(tc.tile_pool(name="acts", bufs=1))
    kt_sb, vt_sb, qt_sb = [], [], []
    for b in range(2):
        kt_sb.append(acts.tile([PDIM, SKV], BF16, name=f"kt_sb{b}"))
        vt_sb.append(acts.tile([PDIM, SKV], BF16, name=f"vt_sb{b}"))
        qt_sb.append(acts.tile([PDIM, S], BF16, name=f"qt_sb{b}"))

    SK0 = min(512, SKV)
    nc.sync.dma_start(wbund[:, 512:1024], i_wb[:, 512:1024])      # wk
    for b in range(2):
        nc.sync.dma_start(kt_sb[b][:, 0:SK0],
                          i_kt[b * PDIM:(b + 1) * PDIM, 0:SK0])
    nc.sync.dma_start(wbund[:, 0:512], i_wb[:, 0:512])            # wq
    for b in range(2):
        nc.sync.dma_start(qt_sb[b][:, 0:512],
                          i_qt[b * PDIM:(b + 1) * PDIM, 0:512])
    nc.sync.dma_start(wbund[:, 1024:1536], i_wb[:, 1024:1536])    # wv
    for b in range(2):
        nc.sync.dma_start(vt_sb[b][:, 0:SK0],
                          i_vt[b * PDIM:(b + 1) * PDIM, 0:SK0])
    bvrow_sb = consts.tile([1, D], BF16, name="bvrow_sb")
    nc.sync.dma_start(bvrow_sb[:], i_bvrow[:])
    if SKV > SK0:
        for b in range(2):
            nc.sync.dma_start(kt_sb[b][:, SK0:SKV],
                              i_kt[b * PDIM:(b + 1) * PDIM, SK0:SKV])
    if SKV > SK0:
        for b in range(2):
            nc.sync.dma_start(vt_sb[b][:, SK0:SKV],
                              i_vt[b * PDIM:(b + 1) * PDIM, SK0:SKV])
    for b in range(2):
        nc.sync.dma_start(qt_sb[b][:, 512:S],
                          i_qt[b * PDIM:(b + 1) * PDIM, 512:S])
    nc.sync.dma_start(wbund[:, 1536:2048], i_wb[:, 1536:2048])    # wo
    nc.sync.dma_start(wbund[:, 2048:2048 + PDIM],
                      i_wb[:, 2048:2048 + PDIM])                  # ident
    onecol = consts.tile([1, PDIM], BF16, name="onecol")
    nc.gpsimd.memset(onecol[:], 1.0)
    # exp table load at t~0, off the critical path of the first real exp
    warm = consts.tile([PDIM, 1], F32, name="warm")
    nc.scalar.activation(warm[:], fb[:, 6:7],
                         mybir.ActivationFunctionType.Exp)

    QT = [acts.tile([PDIM, S], F32R, name=f"QT{b}") for b in range(2)]
    KT = [acts.tile([PDIM, SKV], F32R, name=f"KT{b}") for b in range(2)]
    # V natural layout per kv tile: [128 kv, 8 heads x (32 V | 1 ones)] bf16
    VNE = [acts.tile([PDIM, H * (DH + 1)], BF16, name=f"VNE{j}")
           for j in range(NJ)]
    for j in range(NJ):
        v3 = VNE[j][:].rearrange("p (h c) -> p h c", c=DH + 1)
        nc.gpsimd.memset(v3[:, :, DH:DH + 1], 1.0)

    # --- pools ---
    lt_pool = ctx.enter_context(
        tc.tile_pool(name="lt", bufs=3, space="PSUM"))      # 6 banks
    bank_a = ctx.enter_context(
        tc.tile_pool(name="bnka", bufs=1, space="PSUM"))    # 1 bank
    bank_b = ctx.enter_context(
        tc.tile_pool(name="bnkb", bufs=1, space="PSUM"))    # 1 bank
    p_pool = ctx.enter_context(tc.tile_pool(name="psb", bufs=8))
    onat_pool = ctx.enter_context(tc.tile_pool(name="onat", bufs=3))
    otsb_pool = ctx.enter_context(tc.tile_pool(name="otsb", bufs=2))
    rec_pool = ctx.enter_context(tc.tile_pool(name="recp", bufs=4))
    ft_pool = ctx.enter_context(tc.tile_pool(name="ftp", bufs=4))

    def lt_borrow(cols):
        t = lt_pool.tile([PDIM, 1024], F32, tag="lt", name="ltb")
        return t[:, 0:cols]

    def proj_qk_one(dst, w_sb, b_sb, x_sb, c0, w, ob, ps, eng="D"):
        for ib in range(2):
            nc.tensor.matmul(
                ps[:, 0:w],
                lhsT=w_sb[ib][:, ob * PDIM:(ob + 1) * PDIM],
                rhs=x_sb[ib][:, c0:c0 + w],
                start=(ib == 0), stop=(ib == 1),
            )
        if eng == "D":
            nc.vector.tensor_scalar_add(
                dst[ob][:, c0:c0 + w], ps[:, 0:w], b_sb[:, ob:ob + 1])
        else:
            nc.scalar.activation(
                dst[ob][:, c0:c0 + w], ps[:, 0:w],
                mybir.ActivationFunctionType.Identity,
                bias=b_sb[:, ob:ob + 1])

    def proj_v(j):
        ps = bank_b.tile([PDIM, 512], F32, tag="b", name="bbank")
        for ib in range(2):
            nc.tensor.matmul(
                ps[:, 0:D],
                lhsT=vt_sb[ib][:, j * PDIM:(j + 1) * PDIM],
                rhs=wv_sb[ib][:],
                start=(ib == 0), stop=False,
            )
        nc.tensor.matmul(ps[:, 0:D], lhsT=onecol[:], rhs=bvrow_sb[:],
                         start=False, stop=True)
        v3 = VNE[j][:].rearrange("p (h c) -> p h c", c=DH + 1)
        nc.scalar.activation(
            v3[:, :, 0:DH], ps[:, 0:D].rearrange("p (h c) -> p h c", c=DH),
            mybir.ActivationFunctionType.Copy)

    # K projection (bf16 inputs -> f32r KT); chunk 0 split across the two
    # single-bank pools so logits can start after ~one matmul pass; all other
    # projection chunks borrow lt-ring slots
    kc = chunks(SKV)
    c0, w = kc[0]
    proj_qk_one(KT, wk_sb, bk_sb, kt_sb, c0, w, 0,
                bank_a.tile([PDIM, 512], F32, tag="a", name="abank"))
    proj_qk_one(KT, wk_sb, bk_sb, kt_sb, c0, w, 1,
                bank_b.tile([PDIM, 512], F32, tag="b", name="bbank"), eng="S")
    proj_qk_one(QT, wq_sb, bq_sb, qt_sb, 0, 512, 0, lt_borrow(512))
    proj_qk_one(QT, wq_sb, bq_sb, qt_sb, 0, 512, 1, lt_borrow(512), eng="S")
    # remaining K chunks interleave into the first head-pair group
    deferred = []
    for c0, w in kc[1:]:
        for ob in range(2):
            deferred.append((c0, w, ob))

    exp_engines = _exp_engines(NQC * 4 * NJ)

    def tail_stage0(c, last=False):
        """normalized O_nat[c] -> transpose -> OT sbuf"""
        onat_lo, onat_hi = onat_tiles[c]
        otp = lt_pool.tile([PDIM, 1024], F32, tag="lt")
        otv = otp[:].bitcast(BF16)[:, 0:1024]
        for k in range(8):
            s, fh = k // 2, k % 2
            src_t = onat_lo if fh == 0 else onat_hi
            nc.tensor.matmul(
                otv[:, fh * 512 + s * PDIM:fh * 512 + (s + 1) * PDIM],
                lhsT=src_t[:, s * PDIM:(s + 1) * PDIM],
                rhs=ident, is_transpose=True,
                start=(k == 0), stop=(k == 7), skip_group_check=True)
        ot_sb = otsb_pool.tile([PDIM, 1024], BF16, tag="otsb")
        if last:
            nc.scalar.activation(ot_sb[:, 0:512], otv[:, 0:512],
                                 mybir.ActivationFunctionType.Copy)
            nc.vector.tensor_scalar_add(ot_sb[:, 512:1024],
                                        otv[:, 512:1024], 0.0)
        else:
            nc.scalar.activation(ot_sb[:], otv,
                                 mybir.ActivationFunctionType.Copy)
        return ot_sb

    def tail_proj(c, ot_sb, ob, last=False):
        ps = (bank_a.tile([PDIM, 512], F32, tag="a", name="abank") if ob == 0
              else lt_borrow(512))
        for fb in range(2):
            nc.tensor.matmul(
                ps[:],
                lhsT=wo_sb[fb][:, ob * PDIM:(ob + 1) * PDIM],
                rhs=ot_sb[:, fb * 512:(fb + 1) * 512],
                start=(fb == 0), stop=(fb == 1))
        ft = ft_pool.tile([PDIM, 512], F32, tag="ft")
        if last and ob == 1:
            nc.vector.tensor_scalar_add(ft[:], ps[:], bo_sb[:, ob:ob + 1])
        else:
            nc.scalar.activation(
                ft[:], ps[:], mybir.ActivationFunctionType.Identity,
                bias=bo_sb[:, ob:ob + 1])
        nc.sync.dma_start(
            o_ot[ob * PDIM:(ob + 1) * PDIM, c * 512:(c + 1) * 512], ft[:])

    onat_tiles = {}
    tail_ot = [None]
    eidx = 0
    for ic in range(NQC):
        onat_lo = onat_pool.tile([PDIM, 512], BF16, tag="onlo")
        onat_hi = onat_pool.tile([PDIM, 512], BF16, tag="onhi")
        onat_tiles[ic] = (onat_lo, onat_hi)
        i0 = ic * 512
        for hp in range(4):
            h0 = 2 * hp
            opv = (bank_a.tile([PDIM, 512], F32, tag="a", name="abank") if hp % 2 == 0
                   else bank_b.tile([PDIM, 512], F32, tag="b", name="bbank"))
            first_pv = [True]
            pvq = []

            def emit_pv(j, pt, last):
                for s in range(4):
                    for hh in range(2):
                        g = 2 * s + hh
                        nc.tensor.matmul(
                            opv[:, 33 * g:33 * g + 33],
                            lhsT=pt[:, hh * 512 + s * PDIM:
                                    hh * 512 + (s + 1) * PDIM],
                            rhs=VNE[j][:, 33 * (h0 + hh):33 * (h0 + hh) + 33],
                            start=first_pv[0], stop=last,
                            skip_group_check=True)
                        first_pv[0] = False

            for j in range(NJ):
                if hp == 1 and ic > 0:
                    if j == 0:
                        tail_ot[0] = tail_stage0(ic - 1)
                    elif j == min(2, NJ - 1):
                        tail_proj(ic - 1, tail_ot[0], 0)
                    elif j == min(4, NJ - 1):
                        tail_proj(ic - 1, tail_ot[0], 1)
                if hp == 3 and ic + 1 < NQC and j in (0, min(2, NJ - 1)):
                    ob = 0 if j == 0 else 1
                    proj_qk_one(
                        QT, wq_sb, bq_sb, qt_sb, (ic + 1) * 512, 512, ob,
                        bank_a.tile([PDIM, 512], F32, tag="a",
                                    name="abank") if ob == 0
                        else lt_borrow(512), eng="S")
                if hp == 3 and ic == NQC - 1 and j == 0:
                    # last chunk: transpose+copy the first feature half
                    # (heads 0-3, normalized after hp1) on bank A now
                    otp_l = bank_a.tile([PDIM, 512], F32, tag="a",
                                        name="abank")
                    otv_l = otp_l[:].bitcast(BF16)
                    onat3lo, _ = onat_tiles[ic]
                    for s4 in range(4):
                        nc.tensor.matmul(
                            otv_l[:, s4 * PDIM:(s4 + 1) * PDIM],
                            lhsT=onat3lo[:, s4 * PDIM:(s4 + 1) * PDIM],
                            rhs=ident, is_transpose=True,
                            start=(s4 == 0), stop=False,
                            skip_group_check=True)
                    tail_ot[0] = otsb_pool.tile([PDIM, 1024], BF16,
                                                tag="otsb", name="otsb_t")
                if hp == 3 and ic == NQC - 1 and j == 1:
                    nc.scalar.activation(tail_ot[0][:, 0:512],
                                         otv_l[:, 0:512],
                                         mybir.ActivationFunctionType.Copy)
                # logits for the head pair into one [128, 1024] PSUM tile
                lt = lt_pool.tile([PDIM, 1024], F32, tag="lt", name="ltb")
                for hh, h in enumerate((h0, h0 + 1)):
                    t, bp = h // 4, DH * (h % 4)
                    nc.tensor.matmul(
                        lt[:, hh * 512:(hh + 1) * 512],
                        lhsT=KT[t][bp:bp + DH, j * PDIM:(j + 1) * PDIM],
                        rhs=QT[t][bp:bp + DH, i0:i0 + 512],
                        start=True, stop=True,
                        tile_position=(bp, 0),
                    )
                # exp on the assigned engine
                pt = p_pool.tile([PDIM, 1024], BF16, tag="pt")
                eng = exp_engines[eidx]
                eidx += 1
                if eng == "S":
                    nc.scalar.activation(
                        pt[:], lt[:], mybir.ActivationFunctionType.Exp,
                        bias=kvba_sb[:, j:j + 1], scale=SCALE)
                else:
                    nc.vector.tensor_scalar(
                        pt[:].bitcast(I16), lt[:], SCHRA_A * SCALE,
                        kvbd_sb[:, j:j + 1],
                        op0=mybir.AluOpType.mult, op1=mybir.AluOpType.add)
                if ic == 0 and hp == 0:
                    if deferred and j >= 2:
                        kcc0, kcw, kcob = deferred.pop(0)
                        proj_qk_one(KT, wk_sb, bk_sb, kt_sb, kcc0, kcw, kcob,
                                    lt_borrow(kcw))
                    proj_v(j)
                pvq.append((j, pt))
                if j >= 2:
                    jq, ptq = pvq.pop(0)
                    emit_pv(jq, ptq, last=False)
            while pvq:
                jq, ptq = pvq.pop(0)
                emit_pv(jq, ptq, last=(not pvq))
            # normalize: rec = 1/den (den at col 32 of each 33-group)
            og = opv[:, 0:264].rearrange("p (s h c) -> p s h c", h=2, c=33)
            rec = rec_pool.tile([PDIM, 8], F32, tag="rec")
            r3 = rec[:].rearrange("p (s h) -> p s h", h=2)
            nc.vector.reciprocal(r3.unsqueeze(-1), og[:, :, :, 32:33])
            on_t = onat_lo if hp < 2 else onat_hi
            on = on_t[:].rearrange("p (s f) -> p s f", f=PDIM)
            on = on[:, :, 64 * (hp % 2):64 * (hp % 2 + 1)]
            on = on.rearrange("p s (h c) -> p s h c", c=DH)
            nc.vector.scalar_tensor_tensor(
                on, og[:, :, :, 0:DH], 1.0,
                r3.unsqueeze(-1).broadcast_to((PDIM, 4, 2, DH)),
                op0=mybir.AluOpType.mult, op1=mybir.AluOpType.mult)
    _, onat3hi = onat_tiles[NQC - 1]
    for s4 in range(4):
        nc.tensor.matmul(
            otv_l[:, 512 + s4 * PDIM:512 + (s4 + 1) * PDIM],
            lhsT=onat3hi[:, s4 * PDIM:(s4 + 1) * PDIM],
            rhs=ident, is_transpose=True,
            start=False, stop=(s4 == 3), skip_group_check=True)
    nc.vector.tensor_scalar_add(tail_ot[0][:, 512:1024],
                                otv_l[:, 512:1024], 0.0)
    tail_proj(NQC - 1, tail_ot[0], 0, last=True)
    tail_proj(NQC - 1, tail_ot[0], 1, last=True)


_PROGRAM_CACHE = {}


def _make_program(SKV, S=S):
    nc = bacc.Bacc("TRN2", target_bir_lowering=False, debug=False,
                   enable_asserts=False, num_devices=1)
    NJ = SKV // PDIM
    shapes = dict(qt=(D, S), kt=(D, SKV), vt=(D, SKV),
                  wb=(PDIM, 2176), fb=(PDIM, 6 + 2 * NJ), bvrow=(1, D))
    in_aps = [nc.dram_tensor(k, shapes[k],
                             BF16 if k in _BF16_INPUTS else F32,
                             kind="ExternalInput").ap()
              for k in IN_NAMES]
    out_ap = nc.dram_tensor("ot", (D, S), F32, kind="ExternalOutput").ap()
    with tile.TileContext(nc) as tc:
        _mha_kernel(tc, [out_ap], in_aps, SKV=SKV, S=S)
    nc.compile()
    return nc


def _get_program(SKV):
    if SKV not in _PROGRAM_CACHE:
        _PROGRAM_CACHE[SKV] = _make_program(SKV)
    return _PROGRAM_CACHE[SKV]


def _prepare_in_maps(q, k, v, m, wq, bq, wk, bk, wv, bv, wo, bo):
    mask = np.asarray(m, np.float32).reshape(-1)
    keep = np.flatnonzero(mask == 0.0)
    skv = len(keep)
    assert skv > 0, "all kv positions masked"
    SKV = max(PDIM, ((skv + PDIM - 1) // PDIM) * PDIM)

    NJ = SKV // PDIM
    kvba = np.zeros(SKV, np.float32)
    kvba[skv:] = PAD_BIAS
    kvbd = (SCHRA_A * kvba + SCHRA_B).astype(np.float32)
    bf = ml_dtypes.bfloat16

    def blocks(wm):  # [256, 256] -> [128, 512] (two 128-row blocks side by side)
        a = np.asarray(wm, np.float32)
        return a.reshape(2, PDIM, D).transpose(1, 0, 2).reshape(PDIM, 2 * D)

    wbund = np.concatenate(
        [blocks(wq), blocks(wk), blocks(wv), blocks(wo),
         np.eye(PDIM, dtype=np.float32)], axis=1).astype(bf)
    fbund = np.concatenate(
        [np.asarray(bq, np.float32).reshape(PDIM, 2, order="F"),
         np.asarray(bk, np.float32).reshape(PDIM, 2, order="F"),
         np.asarray(bo, np.float32).reshape(PDIM, 2, order="F"),
         kvba.reshape(NJ, PDIM).T, kvbd.reshape(NJ, PDIM).T], axis=1)
    common = dict(
        wb=np.ascontiguousarray(wbund),
        fb=np.ascontiguousarray(fbund.astype(np.float32)),
        bvrow=np.asarray(bv, np.float32).reshape(1, D).astype(bf),
    )
    in_maps = []
    for b in range(B):
        kg = np.zeros((D, SKV), bf)
        vg = np.zeros((D, SKV), bf)
        kg[:, :skv] = np.asarray(k[b], np.float32).T[:, keep].astype(bf)
        vg[:, :skv] = np.asarray(v[b], np.float32).T[:, keep].astype(bf)
        in_maps.append(dict(
            qt=np.ascontiguousarray(np.asarray(q[b], np.float32).T.astype(bf)),
            kt=kg, vt=vg, **common))
    return in_maps, SKV


def _run(q, k, v, m, wq, bq, wk, bk, wv, bv, wo, bo, trace=False):
    in_maps, SKV = _prepare_in_maps(q, k, v, m, wq, bq, wk, bk, wv, bv, wo, bo)
    nc = _get_program(SKV)
    last_err = None
    for attempt in range(3):
        try:
            res = bass_utils.run_bass_kernel_spmd(
                nc, in_maps, core_ids=list(range(N_CORES)), trace=trace)
            break
        except Exception as e:  # transient device-unrecoverable states heal
            last_err = e        # on the next NEFF load; retry
    else:
        raise last_err
    out = np.stack([res.results[b]["ot"].T for b in range(B)], axis=0)
    return np.ascontiguousarray(out, np.float32), res


def kernel(q, k, v, m, wq, bq, wk, bk, wv, bv, wo, bo):
    out, _ = _run(q, k, v, m, wq, bq, wk, bk, wv, bv, wo, bo, trace=False)
    return out
